# revision 2
# baseline (speedup 1.0000x reference)
"""HebbianConv2d Trainium2 kernel, v2 (split-precision f32r conv).

Full-input contract: kernel(x=(16,256,56,56) f32, weight=(384,256,3,3) f32)
-> (16,384,54,54) f32.  Data-parallel over batch across 8 NeuronCores
(2 samples/core); weights and the lateral-feedback table are replicated.

Key ideas vs v1:
  * Conv runs as THREE f32r matmuls per tap instead of one fp32 matmul:
      y = r(x)*r(w) + r(x)*r(w - r(w)) + r(x - r(x))*r(w)
    where r() is the hardware fp32r rounding (verified on-device to be
    round-to-nearest-even to 11 explicit mantissa bits, identical for
    Act-copy, DVE writes, and both PE operands, and idempotent).  fp32r
    matmuls are ~4x faster per row than fp32 (which runs at 4 cyc/row);
    the 3-term split reproduces fp32-level accuracy (~6e-7 rel, verified
    0 winner flips on the reference inputs), which the winner-take-all
    argmax requires (min top-2 gap is 1.1e-6).
  * w residuals are prepared on the host with an exact rne11 emulation;
    x residuals are computed on-device (Act copy rounds, DVE subtracts).
  * The column-max partition broadcast uses a [1,128]x[1,S] fp32 matmul
    (exact: products v*1.0) instead of a DRAM DMA roundtrip.
  * The lateral-feedback (Gaussian) matmul runs in f32r: the mask is
    exactly representable (0/1) and G only needs ~1e-4 accuracy.
"""
import sys

sys.path.insert(0, "/opt/trn_rl_repo")

import numpy as np

import concourse.bass as bass
import concourse.mybir as mybir
from concourse.bass_utils import run_bass_kernel_spmd

try:
    from tile_fix import TileContextFixed
except ImportError:
    TileContextFixed = None  # defined inline below

if TileContextFixed is None:
    import concourse.tile as tile
    from concourse.vector_clock import ScopedClock, VectorClock

    MAXW = 1

    class TileContextFixed(tile.TileContext):  # noqa: F811
        """Walrus in this container rejects >1 sync-wait per instruction;
        split excess waits onto preceding same-engine nops."""

        _ws_counter = 0

        def _add_instruction(self, inst):
            si = getattr(inst, "sync_info", None)
            eng = getattr(inst, "engine", None)
            if (
                si is not None
                and si.on_wait
                and len(si.on_wait) > MAXW
                and eng is not None
                and eng != mybir.EngineType.Unassigned
            ):
                waits = list(si.on_wait)
                keep, excess = waits[:MAXW], waits[MAXW:]
                while excess:
                    chunk, excess = excess[:MAXW], excess[MAXW:]
                    TileContextFixed._ws_counter += 1
                    nop = mybir.InstNoOp(
                        name=f"{inst.name}-ws{TileContextFixed._ws_counter}",
                        engine=eng,
                        sync_info=mybir.SyncInfo(on_wait=chunk, on_update=[]),
                        bass_nofuse=True,
                    )
                    super()._add_instruction(nop)
                inst.sync_info = mybir.SyncInfo(
                    on_wait=keep, on_update=si.on_update
                )
            super()._add_instruction(inst)

        def _drain_and_barrier(self, tick_clock, wait_clock):
            vc = tick_clock.global_clock
            n = len(vc)
            for proc in range(n):
                t = vc[proc]
                if t <= 0:
                    continue
                v = [0] * n
                v[proc] = t
                nop = self.nc.sync.nop(nofuse=True)
                wait_clock.add_sem_waits(
                    nop.ins, ScopedClock({None: VectorClock(v)})
                )
            self.nc.sync.drain()
            self.nc.all_engine_barrier()
            assert self.sems is not None
            popped = self.nc._tile_sem_poison_stack.pop()
            assert popped is self._sem_poison
            self.nc.clear_and_free_semaphores(
                list(self.sems.allocated().values())
            )
            self.nc.all_engine_barrier()


# Problem constants
B, CIN, COUT, H, W, KS = 16, 256, 384, 56, 56, 3
HOUT = H - KS + 1  # 54
MAP_RADIUS = (COUT - 1) // 2  # 191
LFB_SIGMA = float(MAP_RADIUS)
N_CORES = 8
BPC = B // N_CORES  # samples per core = 2
NCIN = CIN // 128  # 2 cin chunks
NCOUT = COUT // 128  # 3 cout chunks
ROWS_PER_BLOCK = 9
NBLK = HOUT // ROWS_PER_BLOCK  # 6 blocks per sample
SBLK = ROWS_PER_BLOCK * HOUT  # 486 spatial positions per block
NSCH = (SBLK + 127) // 128  # 4 s-subchunks per block for the WTA transposes
DT = mybir.dt.float32
F32R = mybir.dt.float32r


def rne11(v: np.ndarray) -> np.ndarray:
    """Exact emulation of the hardware fp32r rounding: round-to-nearest-
    even to 11 explicit mantissa bits (verified bit-exact on device)."""
    v = np.ascontiguousarray(v, np.float32)
    m = v.view(np.uint32).astype(np.uint64)
    shift = np.uint64(12)
    half = np.uint64(1 << 11)
    low = m & np.uint64((1 << 12) - 1)
    base = m & ~np.uint64((1 << 12) - 1)
    keep = (m >> shift) & np.uint64(1)
    rnd = np.where(
        (low > half) | ((low == half) & (keep == np.uint64(1))),
        base + np.uint64(1 << 12),
        base,
    )
    return rnd.astype(np.uint32).view(np.float32)


def lfb_table() -> np.ndarray:
    """G[j, c] = kern[MAP_RADIUS + j - c], the valid-conv matrix of the
    Gaussian lateral-feedback kernel over the padded channel axis."""
    d = np.abs(np.arange(COUT, dtype=np.float32) - MAP_RADIUS)
    kern = np.exp(-(d.astype(np.float32) ** 2) / np.float32(2.0 * LFB_SIGMA**2))
    kern = kern.astype(np.float32)
    G = np.zeros((COUT, COUT), np.float32)
    for c in range(COUT):
        lo = MAP_RADIUS - c
        G[:, c] = kern[np.clip(np.arange(COUT) + lo, 0, COUT - 1)]
        valid = (np.arange(COUT) + lo >= 0) & (np.arange(COUT) + lo < COUT)
        G[~valid, c] = 0.0
    return G


def build_nc(repeat: int = 1):
    nc = bass.Bass()
    x = nc.declare_dram_parameter("x", [BPC, CIN, H, W], DT, isOutput=False)
    xv = nc.declare_dram_parameter("xv", [BPC, CIN, H, W], F32R, isOutput=False)
    wr = nc.declare_dram_parameter("wr", [CIN, KS, KS, COUT], F32R, isOutput=False)
    we = nc.declare_dram_parameter("we", [CIN, KS, KS, COUT], F32R, isOutput=False)
    g = nc.declare_dram_parameter("g", [COUT, COUT], F32R, isOutput=False)
    ident = nc.declare_dram_parameter("ident", [128, 128], DT, isOutput=False)
    ones = nc.declare_dram_parameter("ones", [NSCH, 128], DT, isOutput=False)
    zrow = nc.declare_dram_parameter("zrow", [NSCH, NSCH * 128], DT, isOutput=False)
    out = nc.declare_dram_parameter(
        "out", [BPC, COUT, HOUT, HOUT], DT, isOutput=True
    )

    XCH = H * W // 4  # residual-prep chunk size (free dim)

    with TileContextFixed(nc) as tc:
        import contextlib

        with contextlib.ExitStack() as ctx:
            consts = ctx.enter_context(tc.tile_pool(name="consts", bufs=1))
            xpool = ctx.enter_context(tc.tile_pool(name="xpool", bufs=2))
            xtpool = ctx.enter_context(tc.tile_pool(name="xtmp", bufs=2))
            ypool = ctx.enter_context(tc.tile_pool(name="ysb", bufs=6))
            mpool = ctx.enter_context(tc.tile_pool(name="msk", bufs=3))
            spool = ctx.enter_context(tc.tile_pool(name="scratch", bufs=2))
            gpool = ctx.enter_context(tc.tile_pool(name="gout", bufs=3))
            yps = ctx.enter_context(
                tc.tile_pool(name="yps", bufs=3, space="PSUM")
            )
            ops = ctx.enter_context(
                tc.tile_pool(name="ops", bufs=2, space="PSUM")
            )
            tps = ctx.enter_context(
                tc.tile_pool(name="tps", bufs=1, space="PSUM")
            )
            rps = ctx.enter_context(
                tc.tile_pool(name="rps", bufs=1, space="PSUM")
            )

            wr_sb = consts.tile([128, NCIN, KS, KS, COUT], F32R)
            we_sb = consts.tile([128, NCIN, KS, KS, COUT], F32R)
            wr_r = wr.rearrange("(c k) kh kw o -> k c kh kw o", k=128)
            we_r = we.rearrange("(c k) kh kw o -> k c kh kw o", k=128)
            for ci in range(NCIN):
                nc.scalar.dma_start(
                    out=wr_sb[:, ci, :, :, :], in_=wr_r[:, ci, :, :, :]
                )
                nc.sync.dma_start(
                    out=we_sb[:, ci, :, :, :], in_=we_r[:, ci, :, :, :]
                )
            g_sb = consts.tile([128, NCOUT, COUT], F32R)
            nc.gpsimd.dma_start(
                out=g_sb[:, :, :],
                in_=g.rearrange("(jc k) c -> k jc c", k=128),
            )
            id_sb = consts.tile([128, 128], DT)
            nc.gpsimd.dma_start(out=id_sb[:, :], in_=ident[:, :])
            ones_sb = consts.tile([NSCH, 128], DT)
            nc.gpsimd.dma_start(out=ones_sb[:, :], in_=ones[:, :])
            zrow_sb = consts.tile([NSCH, NSCH * 128], DT)
            nc.gpsimd.dma_start(out=zrow_sb[:, :], in_=zrow[:, :])

            for _rep in range(repeat):
                for b in range(BPC):
                    # raw x bits, fp32r-labeled: PE rounds the moving
                    # operand to r(x) internally (terms 1 and 2)
                    x_sb = xpool.tile([128, NCIN, H * W], F32R, tag="x")
                    xv_r = xv[b].rearrange("(c k) h w -> k c (h w)", k=128)
                    for ci in range(NCIN):
                        nc.gpsimd.dma_start(
                            out=x_sb[:, ci, :], in_=xv_r[:, ci, :]
                        )
                    # residual xe = x - r(x), computed in chunks
                    xe_sb = xpool.tile([128, NCIN, H * W], F32R, tag="xe")
                    for ci in range(NCIN):
                        for hc in range(4):
                            sl = slice(hc * XCH, (hc + 1) * XCH)
                            x32 = xtpool.tile([128, XCH], DT, tag="x32")
                            nc.gpsimd.dma_start(
                                out=x32[:, :],
                                in_=x[b]
                                .rearrange("(c k) h w -> k c (h w)", k=128)[
                                    :, ci, sl
                                ],
                            )
                            xr_c = xtpool.tile([128, XCH], F32R, tag="xr")
                            nc.scalar.copy(out=xr_c[:, :], in_=x32[:, :])
                            nc.vector.tensor_tensor(
                                out=xe_sb[:, ci, sl],
                                in0=x32[:, :],
                                in1=xr_c[:, :],
                                op=mybir.AluOpType.subtract,
                            )
                    x_hw = [
                        x_sb[:, ci, :].rearrange("k (h w) -> k h w", w=W)
                        for ci in range(NCIN)
                    ]
                    xe_hw = [
                        xe_sb[:, ci, :].rearrange("k (h w) -> k h w", w=W)
                        for ci in range(NCIN)
                    ]
                    for blk in range(NBLK):
                        oh0 = blk * ROWS_PER_BLOCK
                        y_ps = []
                        for cc in range(NCOUT):
                            acc = yps.tile([128, SBLK], DT, tag="ypsum")
                            n_mm = NCIN * KS * KS * 3
                            k = 0
                            for term in range(3):
                                for ci in range(NCIN):
                                    for kh in range(KS):
                                        for kw in range(KS):
                                            xh = (
                                                xe_hw if term == 1 else x_hw
                                            )[ci][
                                                :,
                                                oh0 + kh : oh0 + kh + ROWS_PER_BLOCK,
                                                kw : kw + HOUT,
                                            ]
                                            wsl = (
                                                we_sb if term == 2 else wr_sb
                                            )[
                                                :, ci, kh, kw,
                                                cc * 128 : (cc + 1) * 128,
                                            ]
                                            nc.tensor.matmul(
                                                out=acc[:, :],
                                                lhsT=wsl,
                                                rhs=xh,
                                                start=(k == 0),
                                                stop=(k == n_mm - 1),
                                            )
                                            k += 1
                            y_ps.append(acc)

                        # y chunks PSUM -> SBUF (scalar engine)
                        y_sb = []
                        for cc in range(NCOUT):
                            ysb = ypool.tile([128, SBLK], DT, tag="ysb")
                            nc.scalar.copy(out=ysb[:, :], in_=y_ps[cc][:, :])
                            y_sb.append(ysb)

                        # column max across all 384 channels:
                        # chunk-merge on DVE, per-column max over the 128
                        # partitions via PE transpose + DVE reduce, then
                        # partition-broadcast via ones-matmul (exact fp32)
                        mx = spool.tile([128, SBLK], DT, tag="mx")
                        nc.vector.tensor_tensor(
                            out=mx[:, :],
                            in0=y_sb[0][:, :],
                            in1=y_sb[1][:, :],
                            op=mybir.AluOpType.max,
                        )
                        nc.vector.tensor_tensor(
                            out=mx[:, :],
                            in0=mx[:, :],
                            in1=y_sb[2][:, :],
                            op=mybir.AluOpType.max,
                        )
                        mxT = tps.tile([128, NSCH, 128], DT, tag="mxT")
                        for k in range(NSCH):
                            w_cols = min(128, SBLK - k * 128)
                            nc.tensor.transpose(
                                out=mxT[:w_cols, k, :],
                                in_=mx[:, k * 128 : k * 128 + w_cols],
                                identity=id_sb[:, :],
                            )
                        cmaxT = spool.tile([128, NSCH], DT, tag="cmaxT")
                        for k in range(NSCH):
                            nc.vector.tensor_reduce(
                                out=cmaxT[:, k : k + 1],
                                in_=mxT[:, k, :],
                                axis=mybir.AxisListType.X,
                                op=mybir.AluOpType.max,
                            )
                        rowps = rps.tile([1, NSCH, 128], DT, tag="rowps")
                        for k in range(NSCH):
                            nc.tensor.transpose(
                                out=rowps[0:1, k, :],
                                in_=cmaxT[:, k : k + 1],
                                identity=id_sb[:, :],
                            )
                        row_sb = spool.tile([1, NSCH * 128], DT, tag="rowsb")
                        nc.scalar.copy(out=row_sb[0:1, :], in_=rowps[0:1, :, :])
                        # partition broadcast: [128,1] ones (x) [1,S] row
                        bc_ps = rps.tile([128, SBLK], DT, tag="bc")
                        nc.tensor.matmul(
                            out=bc_ps[:, :],
                            lhsT=ones_sb[0:1, :],
                            rhs=row_sb[0:1, 0:SBLK],
                            start=True,
                            stop=True,
                        )

                        # winner mask (f32r: 0/1 exact), lateral feedback
                        # in f32r, gate, store
                        masks = []
                        for cc in range(NCOUT):
                            msk = mpool.tile([128, SBLK], F32R, tag="mask")
                            nc.vector.tensor_tensor(
                                out=msk[:, :],
                                in0=y_sb[cc][:, :],
                                in1=bc_ps[:, :],
                                op=mybir.AluOpType.is_ge,
                            )
                            masks.append(msk)
                        o2_ps = []
                        for cc in range(NCOUT):
                            o2 = ops.tile([128, SBLK], DT, tag="o2psum")
                            for jc in range(NCOUT):
                                nc.tensor.matmul(
                                    out=o2[:, :],
                                    lhsT=g_sb[
                                        :, jc, cc * 128 : (cc + 1) * 128
                                    ],
                                    rhs=masks[jc][:, :],
                                    start=(jc == 0),
                                    stop=(jc == NCOUT - 1),
                                )
                            o2_ps.append(o2)
                        for cc in range(NCOUT):
                            go = gpool.tile([128, SBLK], DT, tag="gout")
                            nc.vector.scalar_tensor_tensor(
                                out=go[:, :],
                                in0=o2_ps[cc][:, :],
                                scalar=1.0,
                                in1=y_sb[cc][:, :],
                                op0=mybir.AluOpType.min,
                                op1=mybir.AluOpType.mult,
                            )
                            nc.gpsimd.dma_start(
                                out=out[
                                    b,
                                    cc * 128 : (cc + 1) * 128,
                                    oh0 : oh0 + ROWS_PER_BLOCK,
                                    :,
                                ],
                                in_=go[:, :],
                            )
    return nc


_NC_CACHE = {}


def _get_nc(repeat: int = 1):
    if repeat not in _NC_CACHE:
        _NC_CACHE[repeat] = build_nc(repeat)
    return _NC_CACHE[repeat]


def make_in_maps(x, weight):
    x = np.ascontiguousarray(np.asarray(x), dtype=np.float32)
    weight = np.ascontiguousarray(np.asarray(weight), dtype=np.float32)
    w_t = np.ascontiguousarray(weight.transpose(1, 2, 3, 0))
    wr = rne11(w_t)
    we = rne11((w_t.astype(np.float64) - wr).astype(np.float32))
    G = lfb_table()
    eye = np.eye(128, dtype=np.float32)
    ones = np.ones((NSCH, 128), dtype=np.float32)
    zrow = np.zeros((NSCH, NSCH * 128), dtype=np.float32)
    maps = []
    for i in range(N_CORES):
        xs = np.ascontiguousarray(x[i * BPC : (i + 1) * BPC])
        maps.append(
            {
                "x": xs,
                "xv": xs,
                "wr": wr,
                "we": we,
                "g": G,
                "ident": eye,
                "ones": ones,
                "zrow": zrow,
            }
        )
    return maps


def run_sharded(x, weight, repeat: int = 1):
    nc = _get_nc(repeat)
    in_maps = make_in_maps(x, weight)
    res = run_bass_kernel_spmd(nc, in_maps, list(range(N_CORES)))
    out = np.concatenate(
        [res.results[i]["out"] for i in range(N_CORES)], axis=0
    )
    return out


def kernel(x, weight):
    return run_sharded(x, weight, repeat=1)
